# revision 25
# baseline (speedup 1.0000x reference)
"""Trainium2 Bass kernel for nn_Attention (sparse_attention variant).

Computes, for inputs hidden/encoder_outputs [B,S,D], c_t [B,D], W [OUT,3D],
b [OUT], v [OUT]:

    cat       = concat([hidden, broadcast(c_t), encoder_outputs], axis=2)
    energy    = relu(cat @ W.T + b)            # [B, S, OUT]
    attention = energy @ v                     # [B, S]
    out       = softmax(attention, axis=1)

Strategy (8 NeuronCores, data-parallel over batch, 2 batches/core):
  - Split W = [W1 | W2 | W3] over the feature axis.
  - Matmuls run in fp16 (fp32 PSUM accumulate).  The contraction dim f
    must sit on SBUF partitions, so X feeds in transposed via fp16
    scratch in DRAM + HWDGE xbar-transpose loads.  To keep all DMA
    paths busy in parallel, hidden is cast fp32->fp16 by SWDGE
    DRAM->DRAM DMAs while encoder goes through SBUF (sync-ring fp32
    load -> ScalarE cast -> scalar-ring fp16 store).  Per-s-block
    scratch tensors keep Tile's DRAM dependency tracking from
    serializing the pipeline.
  - Main loop per 128-row s-tile: accumulate
        pre[s, o] = X^T-tiles.T @ W^T-tiles
    over 16 f-chunks x 2 PSUM banks; VectorE then adds the broadcast
    c2[b,:] = c_t[b] @ W2.T + b row and does a fused
    relu(pre)*v + row-sum (accum_out) -> attention logits.
  - Softmax over S=2048 per batch: 128x16 tile, DVE free-dim reduce +
    GpSimd partition all-reduce, ScalarE exp, DVE normalize.
"""

import sys
import numpy as np

for _p in ("/opt/trn_rl_repo",):
    if _p not in sys.path:
        sys.path.insert(0, _p)

import concourse.bass as bass
import concourse.bacc as bacc
import concourse.tile as tile
from concourse.tile import add_dep_helper
from concourse import mybir, bass_isa
from concourse.bass_utils import run_bass_kernel_spmd
from concourse.masks import make_identity

F32 = mybir.dt.float32
F16 = mybir.dt.float16
BF16 = mybir.dt.bfloat16
AF = mybir.ActivationFunctionType
ALU = mybir.AluOpType

B, S, D, OUT = 16, 2048, 1024, 1024
N_CORES = 8
B_LOC = B // N_CORES            # batches per core
S_LOC = B_LOC * S               # 4096 rows of X per core
N_ST = S_LOC // 128             # 32 s-tiles per core
ST_PER_B = S // 128             # 16 s-tiles per batch
FC = D // 128                   # 8 feature chunks per tensor
NB = OUT // 512                 # 2 PSUM banks across OUT
SB_ROWS = 1024                  # s-block rows per transposed-load round
N_SB = S_LOC // SB_ROWS         # 4 s-blocks
ST_PER_SB = SB_ROWS // 128      # 8 s-tiles per s-block


def build_nc():
    nc = bacc.Bacc("TRN2", target_bir_lowering=False, debug=False,
                   num_devices=N_CORES, dynamic_dma_scratch_size=32768)

    hid = nc.dram_tensor("hidden", [S_LOC, D], F32, kind="ExternalInput").ap()
    enc = nc.dram_tensor("enc", [S_LOC, D], F32, kind="ExternalInput").ap()
    ct = nc.dram_tensor("ct", [B_LOC, D], F32, kind="ExternalInput").ap()
    Wd = nc.dram_tensor("W", [OUT, 3 * D], F32, kind="ExternalInput").ap()
    bd = nc.dram_tensor("b", [OUT], F32, kind="ExternalInput").ap()
    vd = nc.dram_tensor("v", [OUT], F32, kind="ExternalInput").ap()
    outd = nc.dram_tensor("out", [B_LOC, S], F32, kind="ExternalOutput").ap()
    scr_h = [nc.dram_tensor(f"scr_h{i}", [SB_ROWS, D], F16).ap()
             for i in range(N_SB)]
    scr_e = [nc.dram_tensor(f"scr_e{i}", [SB_ROWS, D], F16).ap()
             for i in range(N_SB)]

    with tile.TileContext(nc) as tc:
        with (
            tc.tile_pool(name="const", bufs=1) as cpool,
            tc.tile_pool(name="wT", bufs=1) as wpool,
            tc.tile_pool(name="wload", bufs=2) as wload,
            tc.tile_pool(name="xT", bufs=2) as xTpool,
            tc.tile_pool(name="scratch", bufs=2) as spool,
            tc.tile_pool(name="sm", bufs=2) as smpool,
            tc.tile_pool(name="ptp", bufs=2, space=bass.MemorySpace.PSUM) as ptp,
            tc.tile_pool(name="eps", bufs=3, space=bass.MemorySpace.PSUM) as eps,
        ):
            # ---- W: fp32 loads on scalar ring + DVE cast + PE transpose ---
            ident = cpool.tile([128, 128], F16)
            make_identity(nc, ident[:])
            ones_k1 = cpool.tile([1, 128], F16)
            nc.vector.memset(ones_k1[:], 1.0)
            att_all = cpool.tile([128, N_ST], F32)   # attention logits

            # wT[:, j, :] holds chunk j of W^T: j in [0,8)=W1, [8,16)=W2,
            # [16,24)=W3; entry [p, j, o] = W[o, j*128+p].
            wT = wpool.tile([128, 3 * FC, OUT], F16)
            w_dmas = []
            HALF_F = 3 * D // 2
            for oc in range(FC):
                for half in range(2):
                    w_nat = wload.tile([128, HALF_F], F32, tag="wnat")
                    w_dma = nc.scalar.dma_start(
                        w_nat[:], Wd[oc * 128:(oc + 1) * 128,
                                     half * HALF_F:(half + 1) * HALF_F])
                    w_dmas.append(w_dma)
                    w_s = wload.tile([128, HALF_F], F16, tag="ws")
                    nc.vector.tensor_copy(w_s[:], w_nat[:])
                    for j in range(3 * FC // 2):
                        jg = half * (3 * FC // 2) + j
                        pt = ptp.tile([128, 128], F16, tag="tp")
                        nc.tensor.transpose(pt[:],
                                            w_s[:, j * 128:(j + 1) * 128],
                                            ident[:])
                        nc.scalar.copy(
                            wT[:, jg, oc * 128:(oc + 1) * 128], pt[:])

            # ---- X: SWDGE DRAM->DRAM casts, yielding to W loads ---------
            for sb in range(N_SB):
                rows = slice(sb * SB_ROWS, (sb + 1) * SB_ROWS)
                gate = w_dmas[min(6 + 2 * sb, 15)]
                ch = nc.gpsimd.dma_start(scr_h[sb][:], hid[rows, :])
                add_dep_helper(ch.ins, gate.ins,
                               reason="X casts yield HBM to W loads")
                ce = nc.gpsimd.dma_start(scr_e[sb][:], enc[rows, :])
                add_dep_helper(ce.ins, gate.ins,
                               reason="X casts yield HBM to W loads")


            # ---- small constants (sync ring + DVE fp16 casts) -------------
            ctT_f = cpool.tile([128, FC, B_LOC], F32)
            for bb in range(B_LOC):
                nc.sync.dma_start(ctT_f[:, :, bb],
                                  ct[bb].rearrange("(fc p) -> p fc", p=128))
            ctT_h = cpool.tile([128, FC, B_LOC], F16)
            nc.vector.tensor_copy(ctT_h[:], ctT_f[:])
            b_f = cpool.tile([1, OUT], F32)
            nc.sync.dma_start(b_f[:], bd[None, :])
            b_h = cpool.tile([1, OUT], F16)
            nc.vector.tensor_copy(b_h[:], b_f[:])
            v_f = cpool.tile([1, OUT], F32)
            nc.sync.dma_start(v_f[:], vd[None, :])
            v_h = cpool.tile([1, OUT], F16)
            nc.vector.tensor_copy(v_h[:], v_f[:])

            # ---- c2[b,:] = c_t[b] @ W2.T + b, broadcast to 128 rows -------
            c2bc_sb = []
            for bb in range(B_LOC):
                c2_ps = eps.tile([1, OUT], F32, tag="eps")
                for ob in range(NB):
                    sl = slice(ob * 512, (ob + 1) * 512)
                    for fc in range(FC):
                        nc.tensor.matmul(c2_ps[:, sl],
                                         ctT_h[:, fc, bb:bb + 1],
                                         wT[:, FC + fc, sl],
                                         start=(fc == 0), stop=False)
                    nc.tensor.matmul(c2_ps[:, sl], ones_k1[:, :1],
                                     b_h[:, sl], start=False, stop=True)
                c2b = cpool.tile([1, OUT], F16, tag=f"c2_{bb}")
                nc.vector.tensor_copy(c2b[:], c2_ps[:])
                c2bc_ps = eps.tile([128, OUT], F32, tag="eps")
                for ob in range(NB):
                    sl = slice(ob * 512, (ob + 1) * 512)
                    nc.tensor.matmul(c2bc_ps[:, sl], ones_k1[:],
                                     c2b[:, sl], start=True, stop=True)
                c2bc = cpool.tile([128, OUT], F16, tag=f"c2bc_{bb}")
                nc.vector.tensor_copy(c2bc[:], c2bc_ps[:])
                c2bc_sb.append(c2bc)

            # vbc[p, o] = v[o] (fp16) for the fused relu*v epilogue
            vbc_ps = eps.tile([128, OUT], F32, tag="eps")
            for ob in range(NB):
                sl = slice(ob * 512, (ob + 1) * 512)
                nc.tensor.matmul(vbc_ps[:, sl], ones_k1[:], v_h[:, sl],
                                 start=True, stop=True)
            vbc = cpool.tile([128, OUT], F16)
            nc.vector.tensor_copy(vbc[:], vbc_ps[:])

            def emit_softmax(bb):
                sl = slice(bb * ST_PER_B, (bb + 1) * ST_PER_B)
                m1 = smpool.tile([128, 1], F32, tag="m1")
                nc.vector.tensor_reduce(m1[:], att_all[:, sl],
                                        axis=mybir.AxisListType.X,
                                        op=ALU.max)
                mall = smpool.tile([128, 1], F32, tag="mall")
                nc.gpsimd.partition_all_reduce(mall[:], m1[:], channels=128,
                                               reduce_op=bass_isa.ReduceOp.max)
                nmall = smpool.tile([128, 1], F32, tag="nmall")
                nc.vector.tensor_scalar_mul(nmall[:], mall[:], -1.0)
                ex = smpool.tile([128, ST_PER_B], F32, tag="ex")
                rs = smpool.tile([128, 1], F32, tag="rs")
                nc.scalar.activation(ex[:], att_all[:, sl], AF.Exp,
                                     bias=nmall[:], accum_out=rs[:])
                tot = smpool.tile([128, 1], F32, tag="tot")
                nc.gpsimd.partition_all_reduce(tot[:], rs[:], channels=128,
                                               reduce_op=bass_isa.ReduceOp.add)
                rec = smpool.tile([128, 1], F32, tag="rec")
                nc.vector.reciprocal(rec[:], tot[:])
                res_t = smpool.tile([128, ST_PER_B], F32, tag="res")
                nc.vector.tensor_scalar_mul(res_t[:], ex[:], rec[:])
                nc.sync.dma_start(
                    outd[bb].rearrange("(stl p) -> p stl", p=128), res_t[:])

            # ---- main loop: transposed loads per s-block, then matmuls ----
            for sb in range(N_SB):
                xT_h = xTpool.tile([128, FC, SB_ROWS], F16, tag="xth")
                xT_e = xTpool.tile([128, FC, SB_ROWS], F16, tag="xte")
                for fc in range(FC):
                    nc.sync.dma_start(xT_h[:, fc, :],
                                      scr_h[sb][:, fc * 128:(fc + 1) * 128],
                                      transpose=True)
                    nc.sync.dma_start(xT_e[:, fc, :],
                                      scr_e[sb][:, fc * 128:(fc + 1) * 128],
                                      transpose=True)

                for stl in range(ST_PER_SB):
                    st = sb * ST_PER_SB + stl
                    b_idx = st // ST_PER_B
                    ssl = slice(stl * 128, (stl + 1) * 128)

                    e_ps = eps.tile([128, OUT], F32, tag="eps")
                    for ob in range(NB):
                        sl = slice(ob * 512, (ob + 1) * 512)
                        for fc in range(FC):
                            nc.tensor.matmul(e_ps[:, sl], xT_h[:, fc, ssl],
                                             wT[:, fc, sl],
                                             start=(fc == 0), stop=False)
                        for fc in range(FC):
                            nc.tensor.matmul(e_ps[:, sl], xT_e[:, fc, ssl],
                                             wT[:, 2 * FC + fc, sl],
                                             start=False, stop=(fc == FC - 1))

                    # pre += c2[b] (broadcast), then
                    # att[st] = sum_o relu(pre) * v  (fused on VectorE)
                    nc.vector.tensor_add(e_ps[:], e_ps[:], c2bc_sb[b_idx][:])
                    relu_out = spool.tile([128, OUT], BF16, tag="relu")
                    nc.vector.scalar_tensor_tensor(
                        relu_out[:], e_ps[:], 0.0, vbc[:],
                        op0=ALU.max, op1=ALU.mult,
                        accum_out=att_all[:, st:st + 1])
                    if st % ST_PER_B == ST_PER_B - 1:
                        emit_softmax(st // ST_PER_B)

    nc.compile()
    return nc


_NC = None


def _get_nc():
    global _NC
    if _NC is None:
        _NC = build_nc()
    return _NC


def _in_maps(hidden, encoder_outputs, c_t, W, b, v):
    hidden = np.ascontiguousarray(hidden, dtype=np.float32)
    encoder_outputs = np.ascontiguousarray(encoder_outputs, dtype=np.float32)
    c_t = np.ascontiguousarray(c_t, dtype=np.float32)
    W = np.ascontiguousarray(W, dtype=np.float32)
    b = np.ascontiguousarray(b, dtype=np.float32)
    v = np.ascontiguousarray(v, dtype=np.float32)
    maps = []
    for i in range(N_CORES):
        bs = slice(i * B_LOC, (i + 1) * B_LOC)
        maps.append({
            "hidden": hidden[bs].reshape(S_LOC, D),
            "enc": encoder_outputs[bs].reshape(S_LOC, D),
            "ct": c_t[bs],
            "W": W, "b": b, "v": v,
        })
    return maps


def run(hidden, encoder_outputs, c_t, W, b, v, trace=False, tmpdir=None):
    nc = _get_nc()
    maps = _in_maps(hidden, encoder_outputs, c_t, W, b, v)
    res = run_bass_kernel_spmd(nc, maps, list(range(N_CORES)), trace=trace,
                               tmpdir=tmpdir)
    out = np.concatenate([res.results[i]["out"] for i in range(N_CORES)],
                         axis=0)
    return out, res


def kernel(hidden, encoder_outputs, c_t, W, b, v):
    out, _ = run(hidden, encoder_outputs, c_t, W, b, v)
    return out


# revision 26
# speedup vs baseline: 1.1959x; 1.1959x over previous
"""Trainium2 Bass kernel for nn_Attention (sparse_attention variant).

Computes, for inputs hidden/encoder_outputs [B,S,D], c_t [B,D], W [OUT,3D],
b [OUT], v [OUT]:

    cat       = concat([hidden, broadcast(c_t), encoder_outputs], axis=2)
    energy    = relu(cat @ W.T + b)            # [B, S, OUT]
    attention = energy @ v                     # [B, S]
    out       = softmax(attention, axis=1)

Strategy (8 NeuronCores, data-parallel over batch, 2 batches/core):
  - Split W = [W1 | W2 | W3] over the feature axis.
  - Matmuls run in fp16 (fp32 PSUM accumulate).  The contraction dim f
    must sit on SBUF partitions, so X feeds in transposed via fp16
    scratch in DRAM + HWDGE xbar-transpose loads.  To keep all DMA
    paths busy in parallel, hidden is cast fp32->fp16 by SWDGE
    DRAM->DRAM DMAs while encoder goes through SBUF (sync-ring fp32
    load -> ScalarE cast -> scalar-ring fp16 store).  Per-s-block
    scratch tensors keep Tile's DRAM dependency tracking from
    serializing the pipeline.
  - Main loop per 128-row s-tile: accumulate
        pre[s, o] = X^T-tiles.T @ W^T-tiles
    over 16 f-chunks x 2 PSUM banks; VectorE then adds the broadcast
    c2[b,:] = c_t[b] @ W2.T + b row and does a fused
    relu(pre)*v + row-sum (accum_out) -> attention logits.
  - Softmax over S=2048 per batch: 128x16 tile, DVE free-dim reduce +
    GpSimd partition all-reduce, ScalarE exp, DVE normalize.
"""

import sys
import numpy as np

for _p in ("/opt/trn_rl_repo",):
    if _p not in sys.path:
        sys.path.insert(0, _p)

import concourse.bass as bass
import concourse.bacc as bacc
import concourse.tile as tile
from concourse.tile import add_dep_helper
from concourse import mybir, bass_isa
from concourse.bass_utils import run_bass_kernel_spmd
from concourse.masks import make_identity

F32 = mybir.dt.float32
F16 = mybir.dt.float16
BF16 = mybir.dt.bfloat16
AF = mybir.ActivationFunctionType
ALU = mybir.AluOpType

B, S, D, OUT = 16, 2048, 1024, 1024
N_CORES = 8
B_LOC = B // N_CORES            # batches per core
S_LOC = B_LOC * S               # 4096 rows of X per core
N_ST = S_LOC // 128             # 32 s-tiles per core
ST_PER_B = S // 128             # 16 s-tiles per batch
FC = D // 128                   # 8 feature chunks per tensor
NB = OUT // 512                 # 2 PSUM banks across OUT
SB_ROWS = 1024                  # s-block rows per transposed-load round
N_SB = S_LOC // SB_ROWS         # 4 s-blocks
ST_PER_SB = SB_ROWS // 128      # 8 s-tiles per s-block


def build_nc():
    nc = bacc.Bacc("TRN2", target_bir_lowering=False, debug=False,
                   num_devices=N_CORES, dynamic_dma_scratch_size=32768)

    hid = nc.dram_tensor("hidden", [S_LOC, D], F32, kind="ExternalInput").ap()
    enc = nc.dram_tensor("enc", [S_LOC, D], F32, kind="ExternalInput").ap()
    ct = nc.dram_tensor("ct", [B_LOC, D], F32, kind="ExternalInput").ap()
    Wd = nc.dram_tensor("W", [OUT, 3 * D], F32, kind="ExternalInput").ap()
    bd = nc.dram_tensor("b", [OUT], F32, kind="ExternalInput").ap()
    vd = nc.dram_tensor("v", [OUT], F32, kind="ExternalInput").ap()
    outd = nc.dram_tensor("out", [B_LOC, S], F32, kind="ExternalOutput").ap()
    scr_h = [nc.dram_tensor(f"scr_h{i}", [SB_ROWS, D], F16).ap()
             for i in range(N_SB)]
    scr_e = [nc.dram_tensor(f"scr_e{i}", [SB_ROWS, D], F16).ap()
             for i in range(N_SB)]

    with tile.TileContext(nc) as tc:
        with (
            tc.tile_pool(name="const", bufs=1) as cpool,
            tc.tile_pool(name="wT", bufs=1) as wpool,
            tc.tile_pool(name="wload", bufs=2) as wload,
            tc.tile_pool(name="xT", bufs=2) as xTpool,
            tc.tile_pool(name="scratch", bufs=2) as spool,
            tc.tile_pool(name="sm", bufs=2) as smpool,
            tc.tile_pool(name="ptp", bufs=2, space=bass.MemorySpace.PSUM) as ptp,
            tc.tile_pool(name="eps", bufs=3, space=bass.MemorySpace.PSUM) as eps,
        ):
            # ---- W: fp32 loads on scalar ring + DVE cast + PE transpose ---
            ident = cpool.tile([128, 128], F16)
            make_identity(nc, ident[:])
            ones_k1 = cpool.tile([1, 128], F16)
            nc.vector.memset(ones_k1[:], 1.0)
            att_all = cpool.tile([128, N_ST], F32)   # attention logits

            # wT[:, j, :] holds chunk j of W^T: j in [0,8)=W1, [8,16)=W2,
            # [16,24)=W3; entry [p, j, o] = W[o, j*128+p].
            wT = wpool.tile([128, 3 * FC, OUT], F16)
            w_dmas = []
            HALF_F = 3 * D // 2
            for oc in range(FC):
                for half in range(2):
                    w_nat = wload.tile([128, HALF_F], F32, tag="wnat")
                    w_dma = nc.scalar.dma_start(
                        w_nat[:], Wd[oc * 128:(oc + 1) * 128,
                                     half * HALF_F:(half + 1) * HALF_F])
                    w_dmas.append(w_dma)
                    w_s = wload.tile([128, HALF_F], F16, tag="ws")
                    nc.vector.tensor_copy(w_s[:], w_nat[:])
                    for j in range(3 * FC // 2):
                        jg = half * (3 * FC // 2) + j
                        pt = ptp.tile([128, 128], F16, tag="tp")
                        nc.tensor.transpose(pt[:],
                                            w_s[:, j * 128:(j + 1) * 128],
                                            ident[:])
                        nc.vector.tensor_copy(
                            wT[:, jg, oc * 128:(oc + 1) * 128], pt[:])

            # ---- X: SWDGE DRAM->DRAM casts, yielding to W loads ---------
            for sb in range(N_SB):
                rows = slice(sb * SB_ROWS, (sb + 1) * SB_ROWS)
                gate = w_dmas[15]
                ch = nc.gpsimd.dma_start(scr_h[sb][:], hid[rows, :])
                add_dep_helper(ch.ins, gate.ins,
                               reason="X casts yield HBM to W loads")
                ce = nc.gpsimd.dma_start(scr_e[sb][:], enc[rows, :])
                add_dep_helper(ce.ins, gate.ins,
                               reason="X casts yield HBM to W loads")


            # ---- small constants (sync ring + DVE fp16 casts) -------------
            ctT_f = cpool.tile([128, FC, B_LOC], F32)
            for bb in range(B_LOC):
                nc.sync.dma_start(ctT_f[:, :, bb],
                                  ct[bb].rearrange("(fc p) -> p fc", p=128))
            ctT_h = cpool.tile([128, FC, B_LOC], F16)
            nc.vector.tensor_copy(ctT_h[:], ctT_f[:])
            b_f = cpool.tile([1, OUT], F32)
            nc.sync.dma_start(b_f[:], bd[None, :])
            b_h = cpool.tile([1, OUT], F16)
            nc.vector.tensor_copy(b_h[:], b_f[:])
            v_f = cpool.tile([1, OUT], F32)
            nc.sync.dma_start(v_f[:], vd[None, :])
            v_h = cpool.tile([1, OUT], F16)
            nc.vector.tensor_copy(v_h[:], v_f[:])

            # ---- c2[b,:] = c_t[b] @ W2.T + b, broadcast to 128 rows -------
            c2bc_sb = []
            for bb in range(B_LOC):
                c2_ps = eps.tile([1, OUT], F32, tag="eps")
                for ob in range(NB):
                    sl = slice(ob * 512, (ob + 1) * 512)
                    for fc in range(FC):
                        nc.tensor.matmul(c2_ps[:, sl],
                                         ctT_h[:, fc, bb:bb + 1],
                                         wT[:, FC + fc, sl],
                                         start=(fc == 0), stop=False)
                    nc.tensor.matmul(c2_ps[:, sl], ones_k1[:, :1],
                                     b_h[:, sl], start=False, stop=True)
                c2b = cpool.tile([1, OUT], F16, tag=f"c2_{bb}")
                nc.vector.tensor_copy(c2b[:], c2_ps[:])
                c2bc_ps = eps.tile([128, OUT], F32, tag="eps")
                for ob in range(NB):
                    sl = slice(ob * 512, (ob + 1) * 512)
                    nc.tensor.matmul(c2bc_ps[:, sl], ones_k1[:],
                                     c2b[:, sl], start=True, stop=True)
                c2bc = cpool.tile([128, OUT], F16, tag=f"c2bc_{bb}")
                nc.vector.tensor_copy(c2bc[:], c2bc_ps[:])
                c2bc_sb.append(c2bc)

            # vbc[p, o] = v[o] (fp16) for the fused relu*v epilogue
            vbc_ps = eps.tile([128, OUT], F32, tag="eps")
            for ob in range(NB):
                sl = slice(ob * 512, (ob + 1) * 512)
                nc.tensor.matmul(vbc_ps[:, sl], ones_k1[:], v_h[:, sl],
                                 start=True, stop=True)
            vbc = cpool.tile([128, OUT], F16)
            nc.vector.tensor_copy(vbc[:], vbc_ps[:])

            def emit_softmax(bb):
                sl = slice(bb * ST_PER_B, (bb + 1) * ST_PER_B)
                m1 = smpool.tile([128, 1], F32, tag="m1")
                nc.vector.tensor_reduce(m1[:], att_all[:, sl],
                                        axis=mybir.AxisListType.X,
                                        op=ALU.max)
                mall = smpool.tile([128, 1], F32, tag="mall")
                nc.gpsimd.partition_all_reduce(mall[:], m1[:], channels=128,
                                               reduce_op=bass_isa.ReduceOp.max)
                nmall = smpool.tile([128, 1], F32, tag="nmall")
                nc.vector.tensor_scalar_mul(nmall[:], mall[:], -1.0)
                ex = smpool.tile([128, ST_PER_B], F32, tag="ex")
                rs = smpool.tile([128, 1], F32, tag="rs")
                nc.scalar.activation(ex[:], att_all[:, sl], AF.Exp,
                                     bias=nmall[:], accum_out=rs[:])
                tot = smpool.tile([128, 1], F32, tag="tot")
                nc.gpsimd.partition_all_reduce(tot[:], rs[:], channels=128,
                                               reduce_op=bass_isa.ReduceOp.add)
                rec = smpool.tile([128, 1], F32, tag="rec")
                nc.vector.reciprocal(rec[:], tot[:])
                res_t = smpool.tile([128, ST_PER_B], F32, tag="res")
                nc.vector.tensor_scalar_mul(res_t[:], ex[:], rec[:])
                nc.sync.dma_start(
                    outd[bb].rearrange("(stl p) -> p stl", p=128), res_t[:])

            # ---- main loop: transposed loads per s-block, then matmuls ----
            for sb in range(N_SB):
                xT_h = xTpool.tile([128, FC, SB_ROWS], F16, tag="xth")
                xT_e = xTpool.tile([128, FC, SB_ROWS], F16, tag="xte")
                for fc in range(FC):
                    nc.sync.dma_start(xT_h[:, fc, :],
                                      scr_h[sb][:, fc * 128:(fc + 1) * 128],
                                      transpose=True)
                    nc.sync.dma_start(xT_e[:, fc, :],
                                      scr_e[sb][:, fc * 128:(fc + 1) * 128],
                                      transpose=True)

                for stl in range(ST_PER_SB):
                    st = sb * ST_PER_SB + stl
                    b_idx = st // ST_PER_B
                    ssl = slice(stl * 128, (stl + 1) * 128)

                    e_ps = eps.tile([128, OUT], F32, tag="eps")
                    for ob in range(NB):
                        sl = slice(ob * 512, (ob + 1) * 512)
                        for fc in range(FC):
                            nc.tensor.matmul(e_ps[:, sl], xT_h[:, fc, ssl],
                                             wT[:, fc, sl],
                                             start=(fc == 0), stop=False)
                        for fc in range(FC):
                            nc.tensor.matmul(e_ps[:, sl], xT_e[:, fc, ssl],
                                             wT[:, 2 * FC + fc, sl],
                                             start=False, stop=(fc == FC - 1))

                    # pre += c2[b] (broadcast), then
                    # att[st] = sum_o relu(pre) * v  (fused on VectorE)
                    nc.vector.tensor_add(e_ps[:], e_ps[:], c2bc_sb[b_idx][:])
                    relu_out = spool.tile([128, OUT], BF16, tag="relu")
                    nc.vector.scalar_tensor_tensor(
                        relu_out[:], e_ps[:], 0.0, vbc[:],
                        op0=ALU.max, op1=ALU.mult,
                        accum_out=att_all[:, st:st + 1])
                    if st % ST_PER_B == ST_PER_B - 1:
                        emit_softmax(st // ST_PER_B)

    nc.compile()
    return nc


_NC = None


def _get_nc():
    global _NC
    if _NC is None:
        _NC = build_nc()
    return _NC


def _in_maps(hidden, encoder_outputs, c_t, W, b, v):
    hidden = np.ascontiguousarray(hidden, dtype=np.float32)
    encoder_outputs = np.ascontiguousarray(encoder_outputs, dtype=np.float32)
    c_t = np.ascontiguousarray(c_t, dtype=np.float32)
    W = np.ascontiguousarray(W, dtype=np.float32)
    b = np.ascontiguousarray(b, dtype=np.float32)
    v = np.ascontiguousarray(v, dtype=np.float32)
    maps = []
    for i in range(N_CORES):
        bs = slice(i * B_LOC, (i + 1) * B_LOC)
        maps.append({
            "hidden": hidden[bs].reshape(S_LOC, D),
            "enc": encoder_outputs[bs].reshape(S_LOC, D),
            "ct": c_t[bs],
            "W": W, "b": b, "v": v,
        })
    return maps


def run(hidden, encoder_outputs, c_t, W, b, v, trace=False, tmpdir=None):
    nc = _get_nc()
    maps = _in_maps(hidden, encoder_outputs, c_t, W, b, v)
    res = run_bass_kernel_spmd(nc, maps, list(range(N_CORES)), trace=trace,
                               tmpdir=tmpdir)
    out = np.concatenate([res.results[i]["out"] for i in range(N_CORES)],
                         axis=0)
    return out, res


def kernel(hidden, encoder_outputs, c_t, W, b, v):
    out, _ = run(hidden, encoder_outputs, c_t, W, b, v)
    return out
